# revision 1
# baseline (speedup 1.0000x reference)
"""nn_Attention_Bank MoE-routing kernel for 8 Trainium2 NeuronCores.

Sharding: pure data parallel (the spec's sharding_hint). B=8 batch elements
are sharded 1-per-core across the 8 NeuronCores; all expert / routing
weights are replicated (they are tiny: a few 64x64 matrices). No
collectives are needed: each core computes routing + all 3 experts +
the routing-weighted sum for its own batch element, and the full output
is just the concatenation of per-core outputs along batch.

Self-contained: shapes are hardcoded, nothing is read from disk.
"""

import os
from functools import partial

import numpy as np

import jax
import jax.numpy as jnp
from jax.sharding import Mesh, PartitionSpec as P
from jax.experimental.shard_map import shard_map

B, C, H, W = 8, 64, 128, 128
E, HEADS, HID = 3, 16, 16
N_CORES = 8


def _l2norm(t):
    n = jnp.sqrt(jnp.sum(t * t, axis=-1, keepdims=True))
    return t / jnp.maximum(n, 1e-12)


def _expert(x, qkv_w, dw_w, proj_w, temp):
    b, c, h, w = x.shape
    d = c // HEADS
    qkv = jnp.einsum('bchw,oc->bohw', x, qkv_w)  # [b,3c,h,w]
    q, k, v = jnp.split(qkv, 3, axis=1)
    rs = lambda t: t.reshape(b, HEADS, d, h * w)
    q, k, v = _l2norm(rs(q)), _l2norm(rs(k)), rs(v)
    attn = jnp.einsum('bhcn,bhdn->bhcd', q, k) * temp  # [b,head,d,d]
    attn = jax.nn.softmax(attn, axis=-1)
    out = jnp.einsum('bhcd,bhdn->bhcn', attn, v).reshape(b, c, h, w)
    out = jax.lax.conv_general_dilated(
        out, dw_w, window_strides=(1, 1), padding='SAME',
        feature_group_count=c, dimension_numbers=('NCHW', 'OIHW', 'NCHW'))
    return jnp.einsum('bchw,oc->bohw', out, proj_w)


def _forward(x, hidden, qkv_w, dw_w, proj_w, temp, r1_w, r1_b, r3_w, r3_b):
    # routing
    x_global = jnp.mean(x, axis=(2, 3), keepdims=True)           # [b,c,1,1]
    cat = jnp.concatenate([x_global, hidden], axis=1)            # [b,c+16,1,1]
    h1 = jnp.einsum('bchw,oc->bohw', cat, r1_w) + r1_b[None, :, None, None]
    hidden_new = jax.nn.gelu(h1, approximate=False)              # [b,16,1,1]
    logit = jax.nn.relu(jnp.einsum('bchw,oc->bohw', hidden_new, r3_w)
                        + r3_b[None, :, None, None])             # [b,E,1,1]
    outs = jnp.stack([_expert(x, qkv_w[i], dw_w[i], proj_w[i], temp[i])
                      for i in range(E)], axis=1)                # [b,E,c,h,w]
    logit5 = logit[..., None]                                    # [b,E,1,1,1]
    out = jnp.sum(outs * logit5, axis=1)                         # [b,c,h,w]
    return out, hidden_new, logit5


_jit_cache = {}


def _get_runner():
    if 'fn' in _jit_cache:
        return _jit_cache['fn']
    devs = jax.devices()
    if len(devs) >= N_CORES:
        mesh = Mesh(np.asarray(devs[:N_CORES]), ('b',))
        rep = P()
        fn = jax.jit(shard_map(
            _forward, mesh=mesh,
            in_specs=(P('b'), P('b'), rep, rep, rep, rep, rep, rep, rep, rep),
            out_specs=(P('b'), P('b'), P('b')),
            check_rep=False,
        ))
    else:  # fallback: single device
        fn = jax.jit(_forward)
    _jit_cache['fn'] = fn
    return fn


def kernel(**inputs):
    fn = _get_runner()
    args = (inputs['x'], inputs['hidden'], inputs['qkv_w'], inputs['dw_w'],
            inputs['proj_w'], inputs['temp'], inputs['r1_w'], inputs['r1_b'],
            inputs['r3_w'], inputs['r3_b'])
    args = tuple(jnp.asarray(np.asarray(a), jnp.float32) for a in args)
    out, hidden_new, logit5 = fn(*args)
    return (np.asarray(out, np.float32),
            np.asarray(hidden_new, np.float32),
            np.asarray(logit5, np.float32))


if __name__ == '__main__':
    rng = np.random.default_rng(0)
    ins = {
        'x': rng.standard_normal((B, C, H, W), np.float32),
        'hidden': rng.standard_normal((B, HID, 1, 1), np.float32),
        'qkv_w': rng.standard_normal((E, 3 * C, C), np.float32) * 0.05,
        'dw_w': rng.standard_normal((E, C, 1, 3, 3), np.float32) * 0.05,
        'proj_w': rng.standard_normal((E, C, C), np.float32) * 0.05,
        'temp': np.ones((E, HEADS, 1, 1), np.float32),
        'r1_w': rng.standard_normal((HID, C + HID), np.float32) * 0.05,
        'r1_b': np.zeros((HID,), np.float32),
        'r3_w': rng.standard_normal((E, HID), np.float32) * 0.05,
        'r3_b': np.zeros((E,), np.float32),
    }
    outs = kernel(**ins)
    print([o.shape for o in outs])
